# revision 10
# baseline (speedup 1.0000x reference)
"""Trainium2 Bass kernel for nn_Actor (scatter + LN/SELU MLP), 8 NeuronCores.

Self-contained: hardcodes all shapes. kernel(**inputs) takes full unsharded
inputs and returns the full [N, 5] float32 output.

Math (per row r):
  m[r]   = 1 if r appears in coords else 0   (scatter membership mask)
  x      = selu(LN128(feats[r])) * m[r]      (exact when ln1_b == 0)
  z      = [jnt_pos, jnt_goal, weights, x] @ W1 + b1        (141 -> 512)
  out[r] = tanh(selu(LN512(z)) @ W2 + b2) * 10

Device mapping:
  - data parallel over 8 cores, 32768 rows each, row-tiles of 128
  - selu(v) = min(lam*alpha*e^v - lam*alpha, lam*relu(v))  [exact for alpha>1]
  - LN normalize folded into ACT Exp via per-partition scale/bias
  - mask folded into the Exp bias as ln(m) (0 / -1e30) and into relu scale
  - mean(z) via an extra W1@1 matmul column; sum(z^2) via ACT Square accum_out
  - rsqrt via bitcast quake seed + 3 Newton steps on DVE (avoids Sqrt
    activation-table thrash; all ACT funcs stay in the exp_and_others set)
  - mask built on device by gpsimd local_scatter of per-partition indices
"""

import math

import numpy as np

import concourse.bass as bass
import concourse.bacc as bacc
import concourse.tile as tile
from concourse import mybir
from concourse.bass_utils import run_bass_kernel_spmd

F32 = mybir.dt.float32
BF16 = mybir.dt.bfloat16
I16 = mybir.dt.int16
I32 = mybir.dt.int32
NP_BF16 = mybir.dt.np(BF16)

N_CORES = 8
N = 262144
C = 128
JNT = 5
HID = 512
DJ = 16          # padded jnt-concat width: 13 real + 1 ones (b1) + 2 zero
RPC = N // N_CORES          # rows per core
TAU = 10.0
EPS = 1e-5
LAM = 1.0507009873554805
ALPHA = 1.6732632423543772
LNLA = math.log(LAM * ALPHA)
BIGNEG = 1e30
QUAKE = 0x5F3759DF

CHUNK = 8   # tiles per DMA chunk
GRP = 4     # tiles per stats batch group (z psum tiles alive at once)
IDXN = 384  # padded per-partition scatter index count

TRACE = False          # set True (e.g. from test.py) to capture neuron-profile
LAST_EXEC_NS = None    # exec_time_ns of the last run when TRACE was on
LAST_TRACE_DIR = None


def _rsqrt_newton(nc, pool, w, g):
    """Emit DVE ops computing rsqrt(w) for an f32 [128, g] tile. Returns tile."""
    ta = pool.tile([128, g], F32, tag="nwt_a")
    tb = pool.tile([128, g], F32, tag="nwt_b")
    tc_ = pool.tile([128, g], F32, tag="nwt_c")
    # quake seed: y0 = bitcast(QUAKE - (bitcast(w) >> 1))
    nc.vector.tensor_scalar(
        out=ta.bitcast(I32), in0=w.bitcast(I32), scalar1=1, scalar2=None,
        op0=mybir.AluOpType.arith_shift_right,
    )
    nc.vector.tensor_scalar(
        out=tb.bitcast(I32), in0=ta.bitcast(I32), scalar1=-1, scalar2=QUAKE,
        op0=mybir.AluOpType.mult, op1=mybir.AluOpType.add,
    )
    y, yn = tb, tc_
    for _ in range(3):
        # t = y*y ; u = -0.5*t*w ; y' = (u + 1.5) * y
        nc.vector.tensor_tensor(out=ta, in0=y, in1=y, op=mybir.AluOpType.mult)
        nc.vector.scalar_tensor_tensor(
            out=ta, in0=ta, scalar=-0.5, in1=w,
            op0=mybir.AluOpType.mult, op1=mybir.AluOpType.mult,
        )
        nc.vector.scalar_tensor_tensor(
            out=yn, in0=ta, scalar=1.5, in1=y,
            op0=mybir.AluOpType.add, op1=mybir.AluOpType.mult,
        )
        y, yn = yn, y
    return y


def build(rpc=RPC, idxn=IDXN, with_b2=False):
    tiles = rpc // 128
    assert tiles % CHUNK == 0 and CHUNK % GRP == 0
    nchunks = tiles // CHUNK

    nc = bacc.Bacc(None, target_bir_lowering=False, debug=False)

    ident_e = nc.dram_tensor("ident", [128, 128], BF16, kind="ExternalInput")
    feats_t = nc.dram_tensor("feats_t", [128, tiles * 128], BF16, kind="ExternalInput")
    jcat_t = nc.dram_tensor("jcat_t", [DJ, tiles * 128], BF16, kind="ExternalInput")
    midx = nc.dram_tensor("midx", [128, idxn], I16, kind="ExternalInput")
    w1x_e = nc.dram_tensor("w1x", [128, HID], BF16, kind="ExternalInput")
    w1j_e = nc.dram_tensor("w1j", [DJ, HID], BF16, kind="ExternalInput")
    w1sx_e = nc.dram_tensor("w1sx", [128, 1], BF16, kind="ExternalInput")
    w1sj_e = nc.dram_tensor("w1sj", [DJ, 1], BF16, kind="ExternalInput")
    w2c_e = nc.dram_tensor("w2c", [128, 4 * JNT], BF16, kind="ExternalInput")
    b2c_e = None
    if with_b2:
        b2c_e = nc.dram_tensor("b2c", [128, GRP * JNT], F32, kind="ExternalInput")
    out_e = nc.dram_tensor("out_t", [128, tiles * JNT], F32, kind="ExternalOutput")

    with tile.TileContext(nc) as tc:
        with (
            tc.tile_pool(name="consts", bufs=1) as consts,
            tc.tile_pool(name="chunks", bufs=2) as chunks,
            tc.tile_pool(name="work", bufs=3) as work,
            tc.tile_pool(name="stats", bufs=2) as stats,
            tc.tile_pool(name="psum", bufs=1, space="PSUM") as psum,
        ):
            # ---- constants ----
            i128 = consts.tile([128, 128], BF16)
            nc.sync.dma_start(out=i128, in_=ident_e[:, :])
            w1x = consts.tile([128, HID], BF16)
            nc.sync.dma_start(out=w1x, in_=w1x_e[:, :])
            w1j = consts.tile([DJ, HID], BF16)
            nc.sync.dma_start(out=w1j, in_=w1j_e[:, :])
            w1sx = consts.tile([128, 1], BF16)
            nc.sync.dma_start(out=w1sx, in_=w1sx_e[:, :])
            w1sj = consts.tile([DJ, 1], BF16)
            nc.sync.dma_start(out=w1sj, in_=w1sj_e[:, :])
            w2c = consts.tile([128, 4 * JNT], BF16)
            nc.sync.dma_start(out=w2c, in_=w2c_e[:, :])
            b2c = None
            if with_b2:
                b2c = consts.tile([128, GRP * JNT], F32)
                nc.sync.dma_start(out=b2c, in_=b2c_e[:, :])

            # ---- mask via local_scatter ----
            midx_sb = consts.tile([128, idxn], I16)
            nc.sync.dma_start(out=midx_sb, in_=midx[:, :])
            onesd = consts.tile([128, idxn], BF16)
            nc.vector.memset(onesd, 1.0)
            maskb = consts.tile([128, tiles], BF16)
            nc.gpsimd.local_scatter(
                out_ap=maskb[:, :], data_ap=onesd[:, :], idxs_ap=midx_sb[:, :],
                channels=128, num_elems=tiles, num_idxs=idxn,
            )
            maskf = consts.tile([128, tiles], F32)
            nc.vector.tensor_copy(out=maskf, in_=maskb)
            # logmA = (m-1)*BIGNEG + LNLA   (== LNLA for present rows, -inf else)
            logmA = consts.tile([128, tiles], F32)
            nc.vector.tensor_scalar(
                out=logmA, in0=maskf, scalar1=1.0, scalar2=BIGNEG,
                op0=mybir.AluOpType.subtract, op1=mybir.AluOpType.mult,
            )
            nc.vector.tensor_scalar(
                out=logmA, in0=logmA, scalar1=LNLA, scalar2=None,
                op0=mybir.AluOpType.add,
            )

            outres = consts.tile([128, tiles * JNT], F32)

            # ---- psum tiles ----
            # z: 4 banks; xT: 1; x2T: 2; grp (m2 out + mucol): 1  -> 8 banks
            zts = [psum.tile([128, HID], F32, tag=f"z{i}", name=f"z{i}") for i in range(GRP)]
            xTp = psum.tile([128, 128], F32, tag="xT")
            x2Tp = [psum.tile([128, HID], F32, tag=f"x2T{i}", name=f"x2T{i}") for i in range(2)]
            grp_ps = psum.tile([128, GRP * JNT + GRP], F32, tag="grp")

            for ch in range(nchunks):
                fchunk = chunks.tile([128, CHUNK * 128], BF16, tag="fchunk")
                nc.sync.dma_start(
                    out=fchunk, in_=feats_t[:, ch * CHUNK * 128:(ch + 1) * CHUNK * 128])
                jchunk = chunks.tile([DJ, CHUNK * 128], BF16, tag="jchunk")
                nc.sync.dma_start(
                    out=jchunk, in_=jcat_t[:, ch * CHUNK * 128:(ch + 1) * CHUNK * 128])

                for g in range(CHUNK // GRP):
                    gt0 = ch * CHUNK + g * GRP   # first global tile of group
                    c0 = g * GRP * 128           # col offset within chunk
                    mcols = slice(gt0, gt0 + GRP)

                    # ---- LN1 stats (batched over GRP tiles) ----
                    g1mv = stats.tile([128, GRP, 2], F32, tag="g1mv")
                    for i in range(GRP):
                        fsl = fchunk[:, c0 + i * 128: c0 + (i + 1) * 128]
                        st6 = work.tile([128, 6], F32, tag="st6")
                        nc.vector.bn_stats(out=st6, in_=fsl)
                        nc.vector.bn_aggr(out=g1mv[:, i, :], in_=st6)
                    w1v = stats.tile([128, GRP], F32, tag="w1v")
                    nc.vector.tensor_scalar(
                        out=w1v, in0=g1mv[:, :, 1], scalar1=EPS, scalar2=None,
                        op0=mybir.AluOpType.add)
                    inv1g = _rsqrt_newton(nc, stats, w1v, GRP)
                    q1 = stats.tile([128, GRP], F32, tag="q1")
                    nc.vector.tensor_tensor(
                        out=q1, in0=g1mv[:, :, 0], in1=inv1g, op=mybir.AluOpType.mult)
                    biasA1g = stats.tile([128, GRP], F32, tag="biasA1g")
                    nc.vector.scalar_tensor_tensor(
                        out=biasA1g, in0=q1, scalar=-1.0, in1=logmA[:, mcols],
                        op0=mybir.AluOpType.mult, op1=mybir.AluOpType.add)
                    scT1g = stats.tile([128, GRP], F32, tag="scT1g")
                    nc.vector.scalar_tensor_tensor(
                        out=scT1g, in0=inv1g, scalar=LAM, in1=maskf[:, mcols],
                        op0=mybir.AluOpType.mult, op1=mybir.AluOpType.mult)
                    mlacg = stats.tile([128, GRP], F32, tag="mlacg")
                    nc.vector.tensor_scalar(
                        out=mlacg, in0=maskf[:, mcols], scalar1=LAM * ALPHA,
                        scalar2=None, op0=mybir.AluOpType.mult)

                    # ---- per-tile: selu1 chain, T1, M1, Sq ----
                    ssqg = stats.tile([128, GRP], F32, tag="ssqg")
                    xT_sbs = []
                    for i in range(GRP):
                        fsl = fchunk[:, c0 + i * 128: c0 + (i + 1) * 128]
                        jsl = jchunk[:, c0 + i * 128: c0 + (i + 1) * 128]
                        A1 = work.tile([128, 128], BF16, tag="A1")
                        nc.scalar.activation(
                            out=A1, in_=fsl, func=mybir.ActivationFunctionType.Exp,
                            bias=biasA1g[:, i:i + 1], scale=inv1g[:, i:i + 1])
                        A1p = work.tile([128, 128], BF16, tag="A1p")
                        nc.vector.tensor_scalar(
                            out=A1p, in0=A1, scalar1=mlacg[:, i:i + 1], scalar2=None,
                            op0=mybir.AluOpType.subtract)
                        t1 = work.tile([128, 128], BF16, tag="t1")
                        nc.vector.tensor_scalar(
                            out=t1, in0=fsl, scalar1=g1mv[:, i, 0:1], scalar2=0.0,
                            op0=mybir.AluOpType.subtract, op1=mybir.AluOpType.max)
                        xbuf = work.tile([128, 128], BF16, tag="xbuf")
                        nc.vector.scalar_tensor_tensor(
                            out=xbuf, in0=t1, scalar=scT1g[:, i:i + 1], in1=A1p,
                            op0=mybir.AluOpType.mult, op1=mybir.AluOpType.min)
                        # T1 transpose via matmul with identity
                        nc.tensor.matmul(xTp, lhsT=xbuf, rhs=i128, start=True, stop=True)
                        xT_sb = work.tile([128, 128], BF16, tag="xT_sb")
                        nc.vector.tensor_copy(out=xT_sb, in_=xTp)
                        xT_sbs.append(xT_sb)
                        # M1: z = xcatT^T @ W1
                        zt = zts[i]
                        nc.tensor.matmul(zt, lhsT=xT_sb, rhs=w1x, start=True, stop=False)
                        nc.tensor.matmul(zt, lhsT=jsl, rhs=w1j, start=False, stop=True)
                        # mean column
                        mu_ap = grp_ps[:, GRP * JNT + i: GRP * JNT + i + 1]
                        nc.tensor.matmul(mu_ap, lhsT=xT_sb, rhs=w1sx, start=True, stop=False)
                        nc.tensor.matmul(mu_ap, lhsT=jsl, rhs=w1sj, start=False, stop=True)
                        # sum(z^2)
                        sqt = work.tile([128, HID], BF16, tag="sqt")
                        nc.scalar.activation(
                            out=sqt, in_=zt, func=mybir.ActivationFunctionType.Square,
                            accum_out=ssqg[:, i:i + 1])

                    # ---- LN2 stats (batched) ----
                    mu2g = stats.tile([128, GRP], F32, tag="mu2g")
                    nc.vector.tensor_scalar(
                        out=mu2g, in0=grp_ps[:, GRP * JNT: GRP * JNT + GRP],
                        scalar1=1.0 / HID, scalar2=None, op0=mybir.AluOpType.mult)
                    m2e = stats.tile([128, GRP], F32, tag="m2e")
                    nc.vector.tensor_tensor(
                        out=m2e, in0=mu2g, in1=mu2g, op=mybir.AluOpType.mult)
                    nc.vector.tensor_scalar(
                        out=m2e, in0=m2e, scalar1=EPS, scalar2=None,
                        op0=mybir.AluOpType.subtract)
                    w2v = stats.tile([128, GRP], F32, tag="w2v")
                    nc.vector.scalar_tensor_tensor(
                        out=w2v, in0=ssqg, scalar=1.0 / HID, in1=m2e,
                        op0=mybir.AluOpType.mult, op1=mybir.AluOpType.subtract)
                    inv2g = _rsqrt_newton(nc, stats, w2v, GRP)
                    q2 = stats.tile([128, GRP], F32, tag="q2")
                    nc.vector.tensor_tensor(
                        out=q2, in0=mu2g, in1=inv2g, op=mybir.AluOpType.mult)
                    biasA2g = stats.tile([128, GRP], F32, tag="biasA2g")
                    nc.vector.tensor_scalar(
                        out=biasA2g, in0=q2, scalar1=-1.0, scalar2=LNLA,
                        op0=mybir.AluOpType.mult, op1=mybir.AluOpType.add)
                    sc2g = stats.tile([128, GRP], F32, tag="sc2g")
                    nc.vector.tensor_scalar(
                        out=sc2g, in0=inv2g, scalar1=LAM, scalar2=None,
                        op0=mybir.AluOpType.mult)

                    # ---- per-tile: selu2, T2, M2 ----
                    for i in range(GRP):
                        zt = zts[i]
                        A2 = work.tile([128, HID], BF16, tag="A2")
                        nc.scalar.activation(
                            out=A2, in_=zt, func=mybir.ActivationFunctionType.Exp,
                            bias=biasA2g[:, i:i + 1], scale=inv2g[:, i:i + 1])
                        A2p = work.tile([128, HID], BF16, tag="A2p")
                        nc.vector.tensor_scalar(
                            out=A2p, in0=A2, scalar1=LAM * ALPHA, scalar2=None,
                            op0=mybir.AluOpType.subtract)
                        t2 = work.tile([128, HID], BF16, tag="t2")
                        nc.vector.tensor_scalar(
                            out=t2, in0=zt, scalar1=mu2g[:, i:i + 1], scalar2=0.0,
                            op0=mybir.AluOpType.subtract, op1=mybir.AluOpType.max)
                        x2 = work.tile([128, HID], BF16, tag="x2")
                        nc.vector.scalar_tensor_tensor(
                            out=x2, in0=t2, scalar=sc2g[:, i:i + 1], in1=A2p,
                            op0=mybir.AluOpType.mult, op1=mybir.AluOpType.min)
                        # T2: transpose x2 in 4 chunks
                        x2p = x2Tp[i % 2]
                        for cc in range(4):
                            nc.tensor.matmul(
                                x2p[:, cc * 128:(cc + 1) * 128],
                                lhsT=x2[:, cc * 128:(cc + 1) * 128],
                                rhs=i128, start=True, stop=True)
                        x2T_sb = work.tile([128, HID], BF16, tag="x2T_sb")
                        nc.vector.tensor_copy(out=x2T_sb, in_=x2p)
                        # M2
                        o_ap = grp_ps[:, i * JNT:(i + 1) * JNT]
                        for cc in range(4):
                            nc.tensor.matmul(
                                o_ap, lhsT=x2T_sb[:, cc * 128:(cc + 1) * 128],
                                rhs=w2c[:, cc * JNT:(cc + 1) * JNT],
                                start=(cc == 0), stop=(cc == 3))

                    # ---- output: tanh * TAU ----
                    if with_b2:
                        nc.vector.tensor_tensor(
                            out=grp_ps[:, :GRP * JNT], in0=grp_ps[:, :GRP * JNT],
                            in1=b2c, op=mybir.AluOpType.add)
                    tanhg = work.tile([128, GRP * JNT], F32, tag="tanhg")
                    nc.scalar.activation(
                        out=tanhg, in_=grp_ps[:, :GRP * JNT],
                        func=mybir.ActivationFunctionType.Tanh)
                    nc.vector.tensor_scalar(
                        out=outres[:, gt0 * JNT:(gt0 + GRP) * JNT], in0=tanhg,
                        scalar1=TAU, scalar2=None, op0=mybir.AluOpType.mult)

            nc.sync.dma_start(out=out_e[:, :], in_=outres)

    nc.compile()
    return nc


_CACHE = {}


def _get_nc(rpc, idxn, with_b2):
    key = (rpc, idxn, with_b2)
    if key not in _CACHE:
        _CACHE[key] = build(rpc, idxn, with_b2)
    return _CACHE[key]


def _prep_core(feats, jcat_T, core, rpc):
    """Host relayout for one core: partition-major feats, transposed jcat."""
    tiles = rpc // 128
    f = feats[core * rpc:(core + 1) * rpc]
    f_t = np.ascontiguousarray(
        f.reshape(tiles, 128, C).transpose(1, 0, 2).reshape(128, tiles * C)
    ).astype(NP_BF16)
    j_t = np.ascontiguousarray(jcat_T[:, core * rpc:(core + 1) * rpc]).astype(NP_BF16)
    return f_t, j_t


def _mask_indices(coords, rpc, idxn):
    """Per-core [128, idxn] int16 per-partition free-axis scatter indices."""
    u = np.unique(coords)
    out = []
    for core in range(N_CORES):
        lo, hi = core * rpc, (core + 1) * rpc
        lu = u[(u >= lo) & (u < hi)] - lo
        p = lu & 127
        f = lu >> 7
        idx = np.full((128, idxn), -1, dtype=np.int16)
        order = np.argsort(p, kind="stable")
        ps, fs = p[order], f[order]
        counts = np.bincount(ps, minlength=128)
        maxc = counts.max() if counts.size else 0
        assert maxc <= idxn, f"bucket overflow: {maxc} > {idxn}"
        start = 0
        for part in range(128):
            cnt = counts[part]
            idx[part, :cnt] = fs[start:start + cnt]
            start += cnt
        out.append(idx)
    return out


def kernel(feats, coords, jnt_pos, jnt_goal, weights,
           ln1_g, ln1_b, W1, b1, ln2_g, ln2_b, W2, b2):
    return _run(feats, coords, jnt_pos, jnt_goal, weights,
                ln1_g, ln1_b, W1, b1, ln2_g, ln2_b, W2, b2, rpc=RPC)


def _run(feats, coords, jnt_pos, jnt_goal, weights,
         ln1_g, ln1_b, W1, b1, ln2_g, ln2_b, W2, b2, rpc):
    n_all = rpc * N_CORES
    feats = np.asarray(feats, dtype=np.float32)
    coords = np.asarray(coords, dtype=np.int32)
    jnt_pos = np.asarray(jnt_pos, dtype=np.float32)
    jnt_goal = np.asarray(jnt_goal, dtype=np.float32)
    weights = np.asarray(weights, dtype=np.float32)
    ln1_g = np.asarray(ln1_g, dtype=np.float32)
    ln1_b = np.asarray(ln1_b, dtype=np.float32)
    W1 = np.asarray(W1, dtype=np.float32)
    b1 = np.asarray(b1, dtype=np.float32)
    ln2_g = np.asarray(ln2_g, dtype=np.float32)
    ln2_b = np.asarray(ln2_b, dtype=np.float32)
    W2 = np.asarray(W2, dtype=np.float32)
    b2 = np.asarray(b2, dtype=np.float32)

    assert feats.shape == (n_all, C) and coords.shape == (n_all,)
    # Fast device path assumes trivial LN affine params (the reference setup
    # uses exactly these); ln1_g folds into W1 rows, others must be trivial.
    assert np.allclose(ln1_b, 0.0), "ln1_b != 0 unsupported"
    assert np.allclose(ln2_g, 1.0) and np.allclose(ln2_b, 0.0), "ln2 affine unsupported"

    with_b2 = not np.allclose(b2, 0.0)
    nc = _get_nc(rpc, IDXN, with_b2)

    # W1 rows: 0:13 jnt part, 13:141 feats part. Fold ln1_g into feats rows.
    W1j = np.zeros((DJ, HID), np.float32)
    W1j[:13] = W1[:13]
    W1j[13] = b1
    W1x = W1[13:141] * ln1_g[:, None]
    w1sj = W1j.sum(axis=1, keepdims=True)
    w1sx = W1x.sum(axis=1, keepdims=True)
    w2c = np.ascontiguousarray(W2.reshape(4, 128, JNT).transpose(1, 0, 2).reshape(128, 4 * JNT))

    jcat_T = np.zeros((DJ, n_all), np.float32)
    jcat_T[0:JNT] = jnt_pos.T
    jcat_T[JNT:2 * JNT] = jnt_goal.T
    jcat_T[2 * JNT:13] = weights.T
    jcat_T[13] = 1.0

    midxs = _mask_indices(coords, rpc, IDXN)

    const_map = {
        "ident": np.eye(128, dtype=np.float32).astype(NP_BF16),
        "w1x": W1x.astype(NP_BF16),
        "w1j": W1j.astype(NP_BF16),
        "w1sx": w1sx.astype(NP_BF16),
        "w1sj": w1sj.astype(NP_BF16),
        "w2c": w2c.astype(NP_BF16),
    }
    if with_b2:
        const_map["b2c"] = np.tile(b2, (128, GRP)).astype(np.float32)

    in_maps = []
    for core in range(N_CORES):
        f_t, j_t = _prep_core(feats, jcat_T, core, rpc)
        m = dict(const_map)
        m["feats_t"] = f_t
        m["jcat_t"] = j_t
        m["midx"] = midxs[core]
        in_maps.append(m)

    global LAST_EXEC_NS, LAST_TRACE_DIR
    import tempfile
    kw = {}
    if TRACE:
        kw = dict(trace=True, tmpdir=tempfile.mkdtemp(prefix="actor_trace_"))
    res = run_bass_kernel_spmd(nc, in_maps, core_ids=list(range(N_CORES)), **kw)
    LAST_EXEC_NS = res.exec_time_ns
    LAST_TRACE_DIR = kw.get("tmpdir")

    tiles = rpc // 128
    out = np.empty((n_all, JNT), np.float32)
    for core in range(N_CORES):
        o = res.results[core]["out_t"]  # [128, tiles*5]
        o = o.reshape(128, tiles, JNT).transpose(1, 0, 2).reshape(rpc, JNT)
        out[core * rpc:(core + 1) * rpc] = o
    return out


# revision 18
# speedup vs baseline: 1.7859x; 1.7859x over previous
"""Trainium2 Bass kernel for nn_Actor (scatter + LN/SELU MLP), 8 NeuronCores.

Self-contained: hardcodes all shapes. kernel(**inputs) takes full unsharded
inputs and returns the full [N, 5] float32 output.

Math (per row r):
  m[r]   = 1 if r appears in coords else 0   (scatter membership mask)
  x      = selu(LN128(feats[r])) * m[r]      (exact when ln1_b == 0)
  z      = [jnt_pos, jnt_goal, weights, x] @ W1 + b1        (141 -> 512)
  out[r] = tanh(selu(LN512(z)) @ W2 + b2) * 10

Device mapping highlights:
  - data parallel over 8 cores, 32768 rows each, row-tiles of 128
  - selu(v) = min(lam*alpha*e^v - lam*alpha, lam*relu(v))  [exact for alpha>1]
  - LN normalize folded into ACT Exp/Relu via per-partition scale/bias
  - mask folded into the Exp bias as ln(m); the masked rows then yield the
    constant -lam*alpha which is corrected by an extra host-built input
    column (jcatT row 14 = lam*alpha*(1-m)) paired with W1 row = colsum(W1x)
  - sum(z) and sum(z^2) come from extra matmul columns: y = xcat @ (V sqrt(L))
    with G = W1aug@W1aug^T = V L V^T, so sum(y^2) = sum(z^2); plus a W1@1
    column for sum(z).  LN2 stats therefore never touch the z PSUM banks and
    batch across 64 tiles.
  - rsqrt via bitcast quake seed + Newton on DVE (keeps every ACT func in the
    single exp_and_others table set: Exp, Relu, Tanh)
  - mask built on device by gpsimd local_scatter of per-partition indices
  - relu branch of selu2 alternates DVE/ACT per tile; the min-combine and the
    selu1 relu run on otherwise-idle GPSIMD
"""

import math

import numpy as np

import concourse.bass as bass
import concourse.bacc as bacc
import concourse.tile as tile
from concourse import mybir
from concourse.bass_utils import run_bass_kernel_spmd

F32 = mybir.dt.float32
BF16 = mybir.dt.bfloat16
I16 = mybir.dt.int16
I32 = mybir.dt.int32
NP_BF16 = mybir.dt.np(BF16)
OP = mybir.AluOpType
AF = mybir.ActivationFunctionType

N_CORES = 8
N = 262144
C = 128
JNT = 5
HID = 512
DJ = 16          # padded jnt width: 13 real + 1 ones (b1) + 1 mask-corr + 1 zero
DIN = DJ + C     # 144 augmented input width
YW = DIN + 1     # y columns: 144 eigen cols + 1 sum(z) column
RPC = N // N_CORES
TAU = 10.0
EPS = 1e-5
LAM = 1.0507009873554805
ALPHA = 1.6732632423543772
LA = LAM * ALPHA
LNLA = math.log(LA)
BIGNEG = 1e30
QUAKE = 0x5F3759DF

CHUNK = 8    # tiles per DMA chunk
GRP = 2      # tiles per z-psum group
IDXN = 384   # padded per-partition scatter index count

TRACE = False
LAST_EXEC_NS = None
LAST_TRACE_DIR = None


def _newton_rsqrt(nc, pool, w, g, iters=3):
    """DVE rsqrt(w) for f32 [128, g]; returns result tile."""
    ta = pool.tile([128, g], F32, tag="nwt_a", name="nwt_a")
    tb = pool.tile([128, g], F32, tag="nwt_b", name="nwt_b")
    tc_ = pool.tile([128, g], F32, tag="nwt_c", name="nwt_c")
    nc.vector.tensor_scalar(out=ta.bitcast(I32), in0=w.bitcast(I32),
                            scalar1=1, scalar2=None, op0=OP.arith_shift_right)
    nc.vector.tensor_scalar(out=tb.bitcast(I32), in0=ta.bitcast(I32),
                            scalar1=-1, scalar2=QUAKE, op0=OP.mult, op1=OP.add)
    y, yn = tb, tc_
    for _ in range(iters):
        nc.vector.tensor_tensor(out=ta, in0=y, in1=y, op=OP.mult)
        nc.vector.scalar_tensor_tensor(out=ta, in0=ta, scalar=-0.5, in1=w,
                                       op0=OP.mult, op1=OP.mult)
        nc.vector.scalar_tensor_tensor(out=yn, in0=ta, scalar=1.5, in1=y,
                                       op0=OP.add, op1=OP.mult)
        y, yn = yn, y
    return y


def build(rpc=RPC, idxn=IDXN, with_b2=False, sgrp=64):
    tiles = rpc // 128
    sgrp = min(sgrp, tiles)
    assert tiles % sgrp == 0 and sgrp % CHUNK == 0 and CHUNK % GRP == 0
    nsg = tiles // sgrp

    nc = bacc.Bacc(None, target_bir_lowering=False, debug=False)

    ident_e = nc.dram_tensor("ident", [128, 128], BF16, kind="ExternalInput")
    feats_t = nc.dram_tensor("feats_t", [128, tiles * 128], BF16, kind="ExternalInput")
    jcat_t = nc.dram_tensor("jcat_t", [DJ, tiles * 128], BF16, kind="ExternalInput")
    midx = nc.dram_tensor("midx", [128, idxn], I16, kind="ExternalInput")
    w1x_e = nc.dram_tensor("w1x", [128, HID], BF16, kind="ExternalInput")
    w1j_e = nc.dram_tensor("w1j", [DJ, HID], BF16, kind="ExternalInput")
    ywx_e = nc.dram_tensor("ywx", [128, YW], BF16, kind="ExternalInput")
    ywj_e = nc.dram_tensor("ywj", [DJ, YW], BF16, kind="ExternalInput")
    w2c_e = nc.dram_tensor("w2c", [128, 4 * JNT], BF16, kind="ExternalInput")
    if with_b2:
        b2c_e = nc.dram_tensor("b2c", [128, GRP * JNT], F32, kind="ExternalInput")
    out_e = nc.dram_tensor("out_t", [128, tiles * JNT], F32, kind="ExternalOutput")

    with tile.TileContext(nc) as tc:
        with (
            tc.tile_pool(name="consts", bufs=1) as consts,
            tc.tile_pool(name="chunks", bufs=CHUNK + 2) as chunks,
            tc.tile_pool(name="sg", bufs=2) as sg,       # per-supergroup buffers
            tc.tile_pool(name="work", bufs=3) as work,   # per-tile buffers
            tc.tile_pool(name="grpw", bufs=2) as grpw,   # per-4-group buffers
            tc.tile_pool(name="psum", bufs=1, space="PSUM") as psum,
        ):
            # ---- constants ----
            i128 = consts.tile([128, 128], BF16)
            nc.sync.dma_start(out=i128, in_=ident_e[:, :])
            w1x = consts.tile([128, HID], BF16)
            nc.sync.dma_start(out=w1x, in_=w1x_e[:, :])
            w1j = consts.tile([DJ, HID], BF16)
            nc.sync.dma_start(out=w1j, in_=w1j_e[:, :])
            ywx = consts.tile([128, YW], BF16)
            nc.sync.dma_start(out=ywx, in_=ywx_e[:, :])
            ywj = consts.tile([DJ, YW], BF16)
            nc.sync.dma_start(out=ywj, in_=ywj_e[:, :])
            w2c = consts.tile([128, 4 * JNT], BF16)
            nc.sync.dma_start(out=w2c, in_=w2c_e[:, :])
            if with_b2:
                b2c = consts.tile([128, GRP * JNT], F32)
                nc.sync.dma_start(out=b2c, in_=b2c_e[:, :])

            # ---- mask ----
            midx_sb = consts.tile([128, idxn], I16)
            nc.sync.dma_start(out=midx_sb, in_=midx[:, :])
            onesd = consts.tile([128, idxn], BF16)
            nc.vector.memset(onesd, 1.0)
            maskb = consts.tile([128, tiles], BF16)
            nc.gpsimd.local_scatter(
                out_ap=maskb[:, :], data_ap=onesd[:, :], idxs_ap=midx_sb[:, :],
                channels=128, num_elems=tiles, num_idxs=idxn)
            maskf = consts.tile([128, tiles], F32)
            nc.vector.tensor_copy(out=maskf, in_=maskb)
            logmA = consts.tile([128, tiles], F32)
            nc.vector.tensor_scalar(out=logmA, in0=maskf, scalar1=1.0,
                                    scalar2=BIGNEG, op0=OP.subtract, op1=OP.mult)
            nc.vector.tensor_scalar(out=logmA, in0=logmA, scalar1=LNLA,
                                    scalar2=None, op0=OP.add)

            outres = consts.tile([128, tiles * JNT], F32)

            # ---- psum ----
            # GRP=2: zm 2 slots x 2 banks, ymega 2 slots x 1 bank,
            # xTm 1 slot x 1 bank, m2out 1 slot x 1 bank  -> 8 banks
            xTm = psum.tile([128, GRP * 128], F32, tag="xTm")

            for sgi in range(nsg):
                st0 = sgi * sgrp   # first tile of supergroup

                # ---------- phase S: feats DMA + LN1 stats ----------
                fchunks = []
                jchunks = []
                for chi in range(sgrp // CHUNK):
                    base = (st0 + chi * CHUNK) * 128
                    fch = chunks.tile([128, CHUNK * 128], BF16, tag="fchunk",
                                      name=f"fch_{sgi}_{chi}", bufs=sgrp // CHUNK + 2)
                    nc.sync.dma_start(out=fch, in_=feats_t[:, base:base + CHUNK * 128])
                    jch = chunks.tile([DJ, CHUNK * 128], BF16, tag="jchunk",
                                      name=f"jch_{sgi}_{chi}", bufs=sgrp // CHUNK + 2)
                    nc.sync.dma_start(out=jch, in_=jcat_t[:, base:base + CHUNK * 128])
                    fchunks.append(fch)
                    jchunks.append(jch)

                def fsl(i):  # [128, 128] feats slice of local tile i
                    return fchunks[i // CHUNK][:, (i % CHUNK) * 128:(i % CHUNK) * 128 + 128]

                def jsl(i):
                    return jchunks[i // CHUNK][:, (i % CHUNK) * 128:(i % CHUNK) * 128 + 128]

                st1 = sg.tile([128, sgrp, 6], F32, tag="st1", name=f"st1_{sgi}")
                for i in range(sgrp):
                    nc.vector.bn_stats(out=st1[:, i, :], in_=fsl(i))

                # ---- LN1 smalls (batched over sgrp) ----
                msum = sg.tile([128, sgrp], F32, tag="msum", name=f"msum_{sgi}")
                nc.vector.tensor_tensor(out=msum, in0=st1[:, :, 1], in1=st1[:, :, 4], op=OP.add)
                mu1g = sg.tile([128, sgrp], F32, tag="mu1g", name=f"mu1g_{sgi}")
                nc.vector.tensor_scalar(out=mu1g, in0=msum, scalar1=0.5, scalar2=None, op0=OP.mult)
                s1 = sg.tile([128, sgrp], F32, tag="s1", name=f"s1_{sgi}")
                nc.vector.tensor_tensor(out=s1, in0=st1[:, :, 2], in1=st1[:, :, 5], op=OP.add)
                dmu = sg.tile([128, sgrp], F32, tag="dmu", name=f"dmu_{sgi}")
                nc.vector.tensor_tensor(out=dmu, in0=st1[:, :, 1], in1=st1[:, :, 4], op=OP.subtract)
                d2 = sg.tile([128, sgrp], F32, tag="d2", name=f"d2_{sgi}")
                nc.vector.tensor_tensor(out=d2, in0=dmu, in1=dmu, op=OP.mult)
                nc.vector.tensor_scalar(out=d2, in0=d2, scalar1=0.25, scalar2=EPS,
                                        op0=OP.mult, op1=OP.add)
                wboth = sg.tile([128, 2 * sgrp], F32, tag="wboth", name=f"wboth_{sgi}")
                nc.vector.scalar_tensor_tensor(out=wboth[:, 0:sgrp], in0=s1,
                                               scalar=1.0 / 128, in1=d2,
                                               op0=OP.mult, op1=OP.add)

                # ---------- selu1 + T1 + y matmuls ----------
                # needs inv1 -> do LN1 newton first (separately from LN2)
                inv1g = _newton_rsqrt(nc, sg, wboth[:, 0:sgrp], sgrp)
                # NOTE: tile object reuse across sgi handled by pool tags
                q1 = sg.tile([128, sgrp], F32, tag="q1", name=f"q1_{sgi}")
                nc.vector.tensor_tensor(out=q1, in0=msum, in1=inv1g, op=OP.mult)
                biasA1g = sg.tile([128, sgrp], F32, tag="biasA1g", name=f"bA1_{sgi}")
                nc.vector.scalar_tensor_tensor(
                    out=biasA1g, in0=q1, scalar=-0.5, in1=logmA[:, st0:st0 + sgrp],
                    op0=OP.mult, op1=OP.add)
                scT1g = sg.tile([128, sgrp], F32, tag="scT1g", name=f"sT1_{sgi}")
                nc.vector.scalar_tensor_tensor(
                    out=scT1g, in0=inv1g, scalar=LAM, in1=maskf[:, st0:st0 + sgrp],
                    op0=OP.mult, op1=OP.mult)

                xT_sbs = []
                yst6 = sg.tile([128, sgrp, 6], F32, tag="yst6", name=f"y6_{sgi}")
                mu2g = sg.tile([128, sgrp], F32, tag="mu2g", name=f"mu2g_{sgi}")
                ymegas = []
                for q in range(sgrp // GRP):
                    ymega = psum.tile([128, GRP * 256], F32, tag="ymega",
                                      name=f"ym_{sgi}_{q}", bufs=2)
                    ymg = ymega.rearrange("p (g c) -> p g c", c=256)
                    ymegas.append(ymega)
                    A1m = grpw.tile([128, GRP * 128], BF16, tag="A1m", name=f"A1m_{sgi}_{q}")
                    for ii in range(GRP):
                        i = q * GRP + ii
                        nc.scalar.activation(
                            out=A1m[:, ii * 128:(ii + 1) * 128], in_=fsl(i), func=AF.Exp,
                            bias=biasA1g[:, i:i + 1], scale=inv1g[:, i:i + 1])
                    nc.vector.tensor_scalar(out=A1m, in0=A1m, scalar1=LA,
                                            scalar2=None, op0=OP.subtract)
                    for ii in range(GRP):
                        i = q * GRP + ii
                        t1 = work.tile([128, 128], BF16, tag="t1", name=f"t1_{sgi}_{q}_{ii}")
                        nc.vector.tensor_scalar(
                            out=t1, in0=fsl(i), scalar1=mu1g[:, i:i + 1], scalar2=0.0,
                            op0=OP.subtract, op1=OP.max)
                        xbuf = work.tile([128, 128], BF16, tag="xbuf", name=f"xb_{sgi}_{q}_{ii}")
                        nc.vector.scalar_tensor_tensor(
                            out=xbuf, in0=t1, scalar=scT1g[:, i:i + 1],
                            in1=A1m[:, ii * 128:(ii + 1) * 128], op0=OP.mult, op1=OP.min)
                        nc.tensor.matmul(xTm[:, ii * 128:(ii + 1) * 128], lhsT=xbuf,
                                         rhs=i128, start=True, stop=True)
                    xT_sb = grpw.tile([128, GRP * 128], BF16, tag="xT_sb", name=f"xTs_{sgi}_{q}", bufs=sgrp // GRP + 2)
                    nc.vector.tensor_copy(out=xT_sb, in_=xTm)
                    xT_sbs.append(xT_sb)
                    # y matmuls + y stats
                    for ii in range(GRP):
                        i = q * GRP + ii
                        y_ap = ymega[:, ii * 256: ii * 256 + YW]
                        nc.tensor.matmul(y_ap, lhsT=xT_sb[:, ii * 128:(ii + 1) * 128],
                                         rhs=ywx, start=True, stop=False)
                        nc.tensor.matmul(y_ap, lhsT=jsl(i), rhs=ywj, start=False, stop=True)
                    for ii in range(GRP):
                        i = q * GRP + ii
                        nc.vector.bn_stats(out=yst6[:, i, :], in_=ymg[:, ii, 0:DIN])
                    nc.vector.tensor_scalar(
                        out=mu2g[:, q * GRP:(q + 1) * GRP],
                        in0=ymg[:, :, DIN], scalar1=1.0 / HID, scalar2=None, op0=OP.mult)

                # ---- LN2 smalls (batched) ----
                def sumsq(st6, cnt, tag):
                    cv = sg.tile([128, sgrp], F32, tag=f"{tag}cv", name=f"{tag}cv_{sgi}")
                    nc.vector.tensor_tensor(out=cv, in0=st6[:, :, 2], in1=st6[:, :, 5], op=OP.add)
                    ms = sg.tile([128, sgrp], F32, tag=f"{tag}ms", name=f"{tag}ms_{sgi}")
                    nc.vector.tensor_tensor(out=ms, in0=st6[:, :, 1], in1=st6[:, :, 4], op=OP.add)
                    dd = sg.tile([128, sgrp], F32, tag=f"{tag}dd", name=f"{tag}dd_{sgi}")
                    nc.vector.tensor_tensor(out=dd, in0=st6[:, :, 1], in1=st6[:, :, 4], op=OP.subtract)
                    nc.vector.tensor_tensor(out=dd, in0=dd, in1=dd, op=OP.mult)
                    nc.vector.tensor_tensor(out=ms, in0=ms, in1=ms, op=OP.mult)
                    nc.vector.tensor_tensor(out=dd, in0=dd, in1=ms, op=OP.add)
                    nc.vector.scalar_tensor_tensor(out=cv, in0=dd, scalar=cnt / 4.0,
                                                   in1=cv, op0=OP.mult, op1=OP.add)
                    return cv
                sqA = sumsq(yst6, DIN, "sqA")
                m2sq = sg.tile([128, sgrp], F32, tag="m2sq", name=f"m2sq_{sgi}")
                nc.vector.tensor_tensor(out=m2sq, in0=mu2g, in1=mu2g, op=OP.mult)
                nc.vector.tensor_scalar(out=m2sq, in0=m2sq, scalar1=EPS, scalar2=None,
                                        op0=OP.subtract)
                nc.vector.scalar_tensor_tensor(out=wboth[:, sgrp:2 * sgrp], in0=sqA,
                                               scalar=1.0 / HID, in1=m2sq,
                                               op0=OP.mult, op1=OP.subtract)
                inv2g = _newton_rsqrt(nc, sg, wboth[:, sgrp:2 * sgrp], sgrp)
                q2 = sg.tile([128, sgrp], F32, tag="q2", name=f"q2_{sgi}")
                nc.vector.tensor_tensor(out=q2, in0=mu2g, in1=inv2g, op=OP.mult)
                biasA2g = sg.tile([128, sgrp], F32, tag="biasA2g", name=f"bA2_{sgi}")
                nc.vector.tensor_scalar(out=biasA2g, in0=q2, scalar1=-1.0, scalar2=LNLA,
                                        op0=OP.mult, op1=OP.add)
                biasB2g = sg.tile([128, sgrp], F32, tag="biasB2g", name=f"bB2_{sgi}")
                nc.vector.tensor_scalar(out=biasB2g, in0=q2, scalar1=-LAM, scalar2=None,
                                        op0=OP.mult)
                sc2g = sg.tile([128, sgrp], F32, tag="sc2g", name=f"sc2g_{sgi}")
                nc.vector.tensor_scalar(out=sc2g, in0=inv2g, scalar1=LAM, scalar2=None,
                                        op0=OP.mult)

                # ---------- phase Z: per 4-tile group ----------
                for q in range(sgrp // GRP):
                    xT_sb = xT_sbs[q]
                    zmega = psum.tile([128, GRP * HID], F32, tag="zmega",
                                      name=f"zm_{sgi}_{q}", bufs=2)
                    A2m = grpw.tile([128, GRP * HID], BF16, tag="A2m", name=f"A2m_{sgi}_{q}")
                    for ii in range(GRP):
                        i = q * GRP + ii
                        zsl = zmega[:, ii * HID:(ii + 1) * HID]
                        nc.tensor.matmul(zsl, lhsT=xT_sb[:, ii * 128:(ii + 1) * 128],
                                         rhs=w1x, start=True, stop=False)
                        nc.tensor.matmul(zsl, lhsT=jsl(i), rhs=w1j, start=False, stop=True)
                        nc.scalar.activation(
                            out=A2m[:, ii * HID:(ii + 1) * HID], in_=zsl, func=AF.Exp,
                            bias=biasA2g[:, i:i + 1], scale=inv2g[:, i:i + 1])
                    nc.vector.tensor_scalar(out=A2m, in0=A2m, scalar1=LA, scalar2=None,
                                            op0=OP.subtract)
                    for ii in range(GRP):
                        i = q * GRP + ii
                        zsl = zmega[:, ii * HID:(ii + 1) * HID]
                        a2sl = A2m[:, ii * HID:(ii + 1) * HID]
                        x2 = work.tile([128, HID], BF16, tag="x2", name=f"x2_{sgi}_{q}_{ii}")
                        b2t = work.tile([128, HID], BF16, tag="b2t", name=f"b2_{sgi}_{q}_{ii}")
                        nc.scalar.activation(
                            out=b2t, in_=zsl, func=AF.Relu,
                            bias=biasB2g[:, i:i + 1], scale=sc2g[:, i:i + 1])
                        nc.vector.tensor_tensor(out=x2, in0=b2t, in1=a2sl, op=OP.min)
                        for cc in range(4):
                            nc.tensor.matmul(
                                zmega[:, ii * HID + cc * 128: ii * HID + (cc + 1) * 128],
                                lhsT=x2[:, cc * 128:(cc + 1) * 128],
                                rhs=i128, start=True, stop=True)
                    x2T_sb = grpw.tile([128, GRP * HID], BF16, tag="x2T_sb",
                                       name=f"x2T_{sgi}_{q}")
                    nc.vector.tensor_copy(out=x2T_sb, in_=zmega)
                    m2out = psum.tile([128, GRP * JNT], F32, tag="m2out",
                                      name=f"m2o_{sgi}_{q}", bufs=1)
                    for ii in range(GRP):
                        for cc in range(4):
                            nc.tensor.matmul(
                                m2out[:, ii * JNT:(ii + 1) * JNT],
                                lhsT=x2T_sb[:, ii * HID + cc * 128: ii * HID + (cc + 1) * 128],
                                rhs=w2c[:, cc * JNT:(cc + 1) * JNT],
                                start=(cc == 0), stop=(cc == 3))
                    if with_b2:
                        nc.vector.tensor_tensor(out=m2out, in0=m2out, in1=b2c, op=OP.add)
                    tanhg = grpw.tile([128, GRP * JNT], F32, tag="tanhg",
                                      name=f"th_{sgi}_{q}")
                    nc.scalar.activation(out=tanhg, in_=m2out, func=AF.Tanh)
                    gt0 = st0 + q * GRP
                    nc.vector.tensor_scalar(
                        out=outres[:, gt0 * JNT:(gt0 + GRP) * JNT], in0=tanhg,
                        scalar1=TAU, scalar2=None, op0=OP.mult)

            nc.sync.dma_start(out=out_e[:, :], in_=outres)

    nc.compile()
    return nc


_CACHE = {}


def _get_nc(rpc, idxn, with_b2):
    key = (rpc, idxn, with_b2)
    if key not in _CACHE:
        _CACHE[key] = build(rpc, idxn, with_b2)
    return _CACHE[key]


def _prep_core(feats, jcat_T, core, rpc):
    tiles = rpc // 128
    f = feats[core * rpc:(core + 1) * rpc]
    f_t = np.ascontiguousarray(
        f.reshape(tiles, 128, C).transpose(1, 0, 2).reshape(128, tiles * C)
    ).astype(NP_BF16)
    j_t = np.ascontiguousarray(jcat_T[:, core * rpc:(core + 1) * rpc]).astype(NP_BF16)
    return f_t, j_t


def _mask_indices(coords, rpc, idxn):
    """Per-core ([128, idxn] int16 indices, [rpc] 0/1 mask)."""
    u = np.unique(coords)
    idxs, masks = [], []
    for core in range(N_CORES):
        lo, hi = core * rpc, (core + 1) * rpc
        lu = u[(u >= lo) & (u < hi)] - lo
        m = np.zeros(rpc, np.float32)
        m[lu] = 1.0
        p = lu & 127
        f = lu >> 7
        idx = np.full((128, idxn), -1, dtype=np.int16)
        order = np.argsort(p, kind="stable")
        ps, fs = p[order], f[order]
        counts = np.bincount(ps, minlength=128)
        assert counts.max(initial=0) <= idxn, f"bucket overflow {counts.max()}"
        start = 0
        for part in range(128):
            cnt = counts[part]
            idx[part, :cnt] = fs[start:start + cnt]
            start += cnt
        idxs.append(idx)
        masks.append(m)
    return idxs, masks


def kernel(feats, coords, jnt_pos, jnt_goal, weights,
           ln1_g, ln1_b, W1, b1, ln2_g, ln2_b, W2, b2):
    return _run(feats, coords, jnt_pos, jnt_goal, weights,
                ln1_g, ln1_b, W1, b1, ln2_g, ln2_b, W2, b2, rpc=RPC)


def _run(feats, coords, jnt_pos, jnt_goal, weights,
         ln1_g, ln1_b, W1, b1, ln2_g, ln2_b, W2, b2, rpc):
    n_all = rpc * N_CORES
    feats = np.asarray(feats, dtype=np.float32)
    coords = np.asarray(coords, dtype=np.int32)
    jnt_pos = np.asarray(jnt_pos, dtype=np.float32)
    jnt_goal = np.asarray(jnt_goal, dtype=np.float32)
    weights = np.asarray(weights, dtype=np.float32)
    ln1_g = np.asarray(ln1_g, dtype=np.float32)
    ln1_b = np.asarray(ln1_b, dtype=np.float32)
    W1 = np.asarray(W1, dtype=np.float32)
    b1 = np.asarray(b1, dtype=np.float32)
    ln2_g = np.asarray(ln2_g, dtype=np.float32)
    ln2_b = np.asarray(ln2_b, dtype=np.float32)
    W2 = np.asarray(W2, dtype=np.float32)
    b2 = np.asarray(b2, dtype=np.float32)

    assert feats.shape == (n_all, C) and coords.shape == (n_all,)
    assert np.allclose(ln1_b, 0.0), "ln1_b != 0 unsupported"
    assert np.allclose(ln1_g, 1.0), "ln1_g != 1 unsupported"
    assert np.allclose(ln2_g, 1.0) and np.allclose(ln2_b, 0.0), "ln2 affine unsupported"

    with_b2 = not np.allclose(b2, 0.0)
    nc = _get_nc(rpc, IDXN, with_b2)

    midxs, masks = _mask_indices(coords, rpc, IDXN)

    W1x = W1[13:141] * ln1_g[:, None]          # [128, 512]
    w2c = np.ascontiguousarray(
        W2.reshape(4, 128, JNT).transpose(1, 0, 2).reshape(128, 4 * JNT))

    const_map = {
        "ident": np.eye(128, dtype=np.float32).astype(NP_BF16),
        "w1x": W1x.astype(NP_BF16),
        "w2c": w2c.astype(NP_BF16),
    }
    if with_b2:
        const_map["b2c"] = np.tile(b2, (128, GRP)).astype(np.float32)

    # bf16-exact mask correction: the device writes x = bf16(-LA) for masked
    # rows; rows 14/15 carry colsum(W1x_bf16) split into bf16 coarse+residual.
    la_dev = float(np.float32(LA).astype(NP_BF16))
    W1x_bf = W1x.astype(NP_BF16).astype(np.float64)
    S = W1x_bf.sum(axis=0)
    S_hi = S.astype(np.float32).astype(NP_BF16).astype(np.float64)
    S_lo = (S - S_hi).astype(np.float32)
    W1j = np.zeros((DJ, HID), np.float32)
    W1j[:13] = W1[:13]
    W1j[13] = b1
    W1j[14] = S_hi
    W1j[15] = S_lo
    W1jq = W1j.astype(NP_BF16).astype(np.float64)
    W1aug = np.vstack([W1jq, W1x_bf])   # [144, 512] bf16-consistent
    G = W1aug @ W1aug.T
    evals, evecs = np.linalg.eigh(G)
    Weig = evecs * np.sqrt(np.maximum(evals, 0.0))[None, :]   # [144, 144]
    w1s = W1aug.sum(axis=1)
    yWa = np.concatenate([Weig, w1s[:, None]], axis=1).astype(np.float32)  # [144,145]
    const_map["w1j"] = W1j.astype(NP_BF16)
    const_map["ywj"] = yWa[0:DJ].astype(NP_BF16)
    const_map["ywx"] = yWa[DJ:DIN].astype(NP_BF16)

    in_maps = []
    for core in range(N_CORES):
        m = dict(const_map)

        jcat_T = np.zeros((DJ, rpc), np.float32)
        r0 = core * rpc
        jcat_T[0:JNT] = jnt_pos[r0:r0 + rpc].T
        jcat_T[JNT:2 * JNT] = jnt_goal[r0:r0 + rpc].T
        jcat_T[2 * JNT:13] = weights[r0:r0 + rpc].T
        jcat_T[13] = 1.0
        jcat_T[14] = la_dev * (1.0 - masks[core])
        jcat_T[15] = la_dev * (1.0 - masks[core])

        tiles = rpc // 128
        f = feats[r0:r0 + rpc]
        m["feats_t"] = np.ascontiguousarray(
            f.reshape(tiles, 128, C).transpose(1, 0, 2).reshape(128, tiles * C)
        ).astype(NP_BF16)
        m["jcat_t"] = np.ascontiguousarray(jcat_T).astype(NP_BF16)
        m["midx"] = midxs[core]
        in_maps.append(m)

    global LAST_EXEC_NS, LAST_TRACE_DIR
    import tempfile
    kw = {}
    if TRACE:
        kw = dict(trace=True, tmpdir=tempfile.mkdtemp(prefix="actor_trace_"))
    res = run_bass_kernel_spmd(nc, in_maps, core_ids=list(range(N_CORES)), **kw)
    LAST_EXEC_NS = res.exec_time_ns
    LAST_TRACE_DIR = kw.get("tmpdir")

    tiles = rpc // 128
    out = np.empty((n_all, JNT), np.float32)
    for core in range(N_CORES):
        o = res.results[core]["out_t"]
        o = o.reshape(128, tiles, JNT).transpose(1, 0, 2).reshape(rpc, JNT)
        out[core * rpc:(core + 1) * rpc] = o
    return out


# revision 25
# speedup vs baseline: 1.9111x; 1.0701x over previous
"""Trainium2 Bass kernel for nn_Actor (scatter + LN/SELU MLP), 8 NeuronCores.

Self-contained: hardcodes all shapes. kernel(**inputs) takes full unsharded
inputs and returns the full [N, 5] float32 output.

Math (per row r):
  m[r]   = 1 if r appears in coords else 0   (scatter membership mask)
  x      = selu(LN128(feats[r])) * m[r]      (exact when ln1_b == 0)
  z      = [jnt_pos, jnt_goal, weights, x] @ W1 + b1        (141 -> 512)
  out[r] = tanh(selu(LN512(z)) @ W2 + b2) * 10

Device mapping highlights:
  - data parallel over 8 cores, 32768 rows each, row-tiles of 128
  - selu(v) = min(lam*alpha*e^v - lam*alpha, lam*relu(v))  [exact for alpha>1]
  - LN normalize folded into ACT Exp/Relu via per-partition scale/bias
  - mask folded into the Exp bias as ln(m); the masked rows then yield the
    constant -lam*alpha which is corrected by an extra host-built input
    column (jcatT row 14 = lam*alpha*(1-m)) paired with W1 row = colsum(W1x)
  - sum(z) and sum(z^2) come from extra matmul columns: y = xcat @ (V sqrt(L))
    with G = W1aug@W1aug^T = V L V^T, so sum(y^2) = sum(z^2); plus a W1@1
    column for sum(z).  LN2 stats therefore never touch the z PSUM banks and
    batch across 64 tiles.
  - rsqrt via bitcast quake seed + Newton on DVE (keeps every ACT func in the
    single exp_and_others table set: Exp, Relu, Tanh)
  - mask built on device by gpsimd local_scatter of per-partition indices
  - relu branch of selu2 alternates DVE/ACT per tile; the min-combine and the
    selu1 relu run on otherwise-idle GPSIMD
"""

import math

import numpy as np

import concourse.bass as bass
import concourse.bacc as bacc
import concourse.tile as tile
from concourse import mybir
from concourse.bass_utils import run_bass_kernel_spmd

F32 = mybir.dt.float32
BF16 = mybir.dt.bfloat16
I16 = mybir.dt.int16
I32 = mybir.dt.int32
NP_BF16 = mybir.dt.np(BF16)
OP = mybir.AluOpType
AF = mybir.ActivationFunctionType

N_CORES = 8
N = 262144
C = 128
JNT = 5
HID = 512
DJ = 16          # padded jnt width: 13 real + 1 ones (b1) + 1 mask-corr + 1 zero
DIN = DJ + C     # 144 augmented input width
YW = DIN + 1     # y columns: 144 eigen cols + 1 sum(z) column
RPC = N // N_CORES
TAU = 10.0
EPS = 1e-5
LAM = 1.0507009873554805
ALPHA = 1.6732632423543772
LA = LAM * ALPHA
LNLA = math.log(LA)
BIGNEG = 1e30
QUAKE = 0x5F3759DF

CHUNK = 8    # tiles per DMA chunk
GRP = 2      # tiles per z-psum group
IDXN = 384   # padded per-partition scatter index count

TRACE = False
LAST_EXEC_NS = None
LAST_TRACE_DIR = None


def _newton_rsqrt(nc, pool, w, g, iters=3):
    """DVE rsqrt(w) for f32 [128, g]; returns result tile."""
    ta = pool.tile([128, g], F32, tag="nwt_a", name="nwt_a")
    tb = pool.tile([128, g], F32, tag="nwt_b", name="nwt_b")
    tc_ = pool.tile([128, g], F32, tag="nwt_c", name="nwt_c")
    nc.vector.tensor_scalar(out=ta.bitcast(I32), in0=w.bitcast(I32),
                            scalar1=1, scalar2=None, op0=OP.arith_shift_right)
    nc.vector.tensor_scalar(out=tb.bitcast(I32), in0=ta.bitcast(I32),
                            scalar1=-1, scalar2=QUAKE, op0=OP.mult, op1=OP.add)
    y, yn = tb, tc_
    for _ in range(iters):
        nc.vector.tensor_tensor(out=ta, in0=y, in1=y, op=OP.mult)
        nc.vector.scalar_tensor_tensor(out=ta, in0=ta, scalar=-0.5, in1=w,
                                       op0=OP.mult, op1=OP.mult)
        nc.vector.scalar_tensor_tensor(out=yn, in0=ta, scalar=1.5, in1=y,
                                       op0=OP.add, op1=OP.mult)
        y, yn = yn, y
    return y


def build(rpc=RPC, idxn=IDXN, with_b2=False, sgrp=64):
    tiles = rpc // 128
    sgrp = min(sgrp, tiles)
    assert tiles % sgrp == 0 and sgrp % CHUNK == 0 and CHUNK % GRP == 0
    nsg = tiles // sgrp

    nc = bacc.Bacc(None, target_bir_lowering=False, debug=False)

    ident_e = nc.dram_tensor("ident", [128, 128], BF16, kind="ExternalInput")
    feats_t = nc.dram_tensor("feats_t", [128, tiles * 128], BF16, kind="ExternalInput")
    jcat_t = nc.dram_tensor("jcat_t", [64, tiles * 128], BF16, kind="ExternalInput")
    midx = nc.dram_tensor("midx", [128, idxn], I16, kind="ExternalInput")
    w1x_e = nc.dram_tensor("w1x", [128, HID], BF16, kind="ExternalInput")
    w1j_e = nc.dram_tensor("w1j", [64, HID], BF16, kind="ExternalInput")
    ywx_e = nc.dram_tensor("ywx", [128, YW], BF16, kind="ExternalInput")
    ywj_e = nc.dram_tensor("ywj", [64, YW], BF16, kind="ExternalInput")
    w2c_e = nc.dram_tensor("w2c", [128, 4 * JNT], BF16, kind="ExternalInput")
    if with_b2:
        b2c_e = nc.dram_tensor("b2c", [128, GRP * JNT], F32, kind="ExternalInput")
    out_e = nc.dram_tensor("out_t", [128, tiles * JNT], F32, kind="ExternalOutput")

    with tile.TileContext(nc) as tc:
        with (
            tc.tile_pool(name="consts", bufs=1) as consts,
            tc.tile_pool(name="chunks", bufs=CHUNK + 2) as chunks,
            tc.tile_pool(name="sg", bufs=2) as sg,       # per-supergroup buffers
            tc.tile_pool(name="work", bufs=3) as work,   # per-tile buffers
            tc.tile_pool(name="grpw", bufs=2) as grpw,   # per-4-group buffers
            tc.tile_pool(name="psum", bufs=1, space="PSUM") as psum,
        ):
            # ---- constants ----
            i128 = consts.tile([128, 128], BF16)
            nc.sync.dma_start(out=i128, in_=ident_e[:, :])
            w1x = consts.tile([128, HID], BF16)
            nc.sync.dma_start(out=w1x, in_=w1x_e[:, :])
            w1j = consts.tile([64, HID], BF16)
            nc.sync.dma_start(out=w1j, in_=w1j_e[:, :])
            ywx = consts.tile([128, YW], BF16)
            nc.sync.dma_start(out=ywx, in_=ywx_e[:, :])
            ywj = consts.tile([64, YW], BF16)
            nc.sync.dma_start(out=ywj, in_=ywj_e[:, :])
            w2c = consts.tile([128, 4 * JNT], BF16)
            nc.sync.dma_start(out=w2c, in_=w2c_e[:, :])
            if with_b2:
                b2c = consts.tile([128, GRP * JNT], F32)
                nc.sync.dma_start(out=b2c, in_=b2c_e[:, :])

            # ---- mask ----
            midx_sb = consts.tile([128, idxn], I16)
            nc.sync.dma_start(out=midx_sb, in_=midx[:, :])
            onesd = consts.tile([128, idxn], BF16)
            nc.vector.memset(onesd, 1.0)
            maskb = consts.tile([128, tiles], BF16)
            nc.gpsimd.local_scatter(
                out_ap=maskb[:, :], data_ap=onesd[:, :], idxs_ap=midx_sb[:, :],
                channels=128, num_elems=tiles, num_idxs=idxn)
            maskf = consts.tile([128, tiles], F32)
            nc.vector.tensor_copy(out=maskf, in_=maskb)
            logmA = consts.tile([128, tiles], F32)
            nc.vector.tensor_scalar(out=logmA, in0=maskf, scalar1=1.0,
                                    scalar2=BIGNEG, op0=OP.subtract, op1=OP.mult)
            nc.vector.tensor_scalar(out=logmA, in0=logmA, scalar1=LNLA,
                                    scalar2=None, op0=OP.add)

            outres = consts.tile([128, tiles * JNT], F32)

            # ---- psum ----
            # GRP=2: zm 2 slots x 2 banks, ymega 2 slots x 1 bank,
            # xTm 1 slot x 1 bank, m2out 1 slot x 1 bank  -> 8 banks
            xTm = psum.tile([128, GRP * 128], F32, tag="xTm")

            for sgi in range(nsg):
                st0 = sgi * sgrp   # first tile of supergroup

                # ---------- phase S: feats DMA + LN1 stats ----------
                fchunks = []
                jchunks = []
                for chi in range(sgrp // CHUNK):
                    base = (st0 + chi * CHUNK) * 128
                    fch = chunks.tile([128, CHUNK * 128], BF16, tag="fchunk",
                                      name=f"fch_{sgi}_{chi}", bufs=sgrp // CHUNK + 2)
                    nc.sync.dma_start(out=fch, in_=feats_t[:, base:base + CHUNK * 128])
                    jch = chunks.tile([64, CHUNK * 128], BF16, tag="jchunk",
                                      name=f"jch_{sgi}_{chi}", bufs=sgrp // CHUNK + 2)
                    nc.sync.dma_start(out=jch, in_=jcat_t[:, base:base + CHUNK * 128])
                    fchunks.append(fch)
                    jchunks.append(jch)

                def fsl(i):  # [128, 128] feats slice of local tile i
                    return fchunks[i // CHUNK][:, (i % CHUNK) * 128:(i % CHUNK) * 128 + 128]

                def jsl(i, s):
                    c = jchunks[i // CHUNK]
                    return c[32 * s:32 * s + 16, (i % CHUNK) * 128:(i % CHUNK) * 128 + 128]

                st1 = sg.tile([128, sgrp, 6], F32, tag="st1", name=f"st1_{sgi}")
                for i in range(sgrp):
                    nc.vector.bn_stats(out=st1[:, i, :], in_=fsl(i))

                # ---- LN1 smalls (batched over sgrp) ----
                msum = sg.tile([128, sgrp], F32, tag="msum", name=f"msum_{sgi}")
                nc.vector.tensor_tensor(out=msum, in0=st1[:, :, 1], in1=st1[:, :, 4], op=OP.add)
                mu1g = sg.tile([128, sgrp], F32, tag="mu1g", name=f"mu1g_{sgi}")
                nc.vector.tensor_scalar(out=mu1g, in0=msum, scalar1=0.5, scalar2=None, op0=OP.mult)
                s1 = sg.tile([128, sgrp], F32, tag="s1", name=f"s1_{sgi}")
                nc.vector.tensor_tensor(out=s1, in0=st1[:, :, 2], in1=st1[:, :, 5], op=OP.add)
                dmu = sg.tile([128, sgrp], F32, tag="dmu", name=f"dmu_{sgi}")
                nc.vector.tensor_tensor(out=dmu, in0=st1[:, :, 1], in1=st1[:, :, 4], op=OP.subtract)
                d2 = sg.tile([128, sgrp], F32, tag="d2", name=f"d2_{sgi}")
                nc.vector.tensor_tensor(out=d2, in0=dmu, in1=dmu, op=OP.mult)
                nc.vector.tensor_scalar(out=d2, in0=d2, scalar1=0.25, scalar2=EPS,
                                        op0=OP.mult, op1=OP.add)
                wboth = sg.tile([128, 2 * sgrp], F32, tag="wboth", name=f"wboth_{sgi}")
                nc.vector.scalar_tensor_tensor(out=wboth[:, 0:sgrp], in0=s1,
                                               scalar=1.0 / 128, in1=d2,
                                               op0=OP.mult, op1=OP.add)

                # ---------- selu1 + T1 + y matmuls ----------
                # needs inv1 -> do LN1 newton first (separately from LN2)
                inv1g = _newton_rsqrt(nc, sg, wboth[:, 0:sgrp], sgrp)
                # NOTE: tile object reuse across sgi handled by pool tags
                q1 = sg.tile([128, sgrp], F32, tag="q1", name=f"q1_{sgi}")
                nc.vector.tensor_tensor(out=q1, in0=msum, in1=inv1g, op=OP.mult)
                biasA1g = sg.tile([128, sgrp], F32, tag="biasA1g", name=f"bA1_{sgi}")
                nc.vector.scalar_tensor_tensor(
                    out=biasA1g, in0=q1, scalar=-0.5, in1=logmA[:, st0:st0 + sgrp],
                    op0=OP.mult, op1=OP.add)
                scT1g = sg.tile([128, sgrp], F32, tag="scT1g", name=f"sT1_{sgi}")
                nc.vector.scalar_tensor_tensor(
                    out=scT1g, in0=inv1g, scalar=LAM, in1=maskf[:, st0:st0 + sgrp],
                    op0=OP.mult, op1=OP.mult)

                xT_sbs = []
                yst6 = sg.tile([128, sgrp, 6], F32, tag="yst6", name=f"y6_{sgi}")
                mu2g = sg.tile([128, sgrp], F32, tag="mu2g", name=f"mu2g_{sgi}")
                for chi in range(sgrp // CHUNK):
                    t0 = chi * CHUNK
                    fch = fchunks[chi]
                    fch3 = fch.rearrange("p (t c) -> p t c", c=128)
                    A1m = grpw.tile([128, CHUNK * 128], BF16, tag="A1m",
                                    name=f"A1m_{sgi}_{chi}", bufs=3)
                    for k in range(CHUNK):
                        i = t0 + k
                        nc.scalar.activation(
                            out=A1m[:, k * 128:(k + 1) * 128], in_=fch3[:, k, :], func=AF.Exp,
                            bias=biasA1g[:, i:i + 1], scale=inv1g[:, i:i + 1])
                    nc.vector.tensor_scalar(out=A1m, in0=A1m, scalar1=LA,
                                            scalar2=None, op0=OP.subtract)
                    mu1bc = mu1g[:, t0:t0 + CHUNK].rearrange(
                        "p (t one) -> p t one", one=1).broadcast_to([128, CHUNK, 128])
                    sc1bc = scT1g[:, t0:t0 + CHUNK].rearrange(
                        "p (t one) -> p t one", one=1).broadcast_to([128, CHUNK, 128])
                    xmega = grpw.tile([128, CHUNK * 128], BF16, tag="xmega",
                                      name=f"xm_{sgi}_{chi}", bufs=3)
                    xm3 = xmega.rearrange("p (t c) -> p t c", c=128)
                    nc.vector.tensor_tensor(out=xm3, in0=fch3, in1=mu1bc, op=OP.subtract)
                    nc.vector.tensor_scalar(out=xmega, in0=xmega, scalar1=0.0,
                                            scalar2=None, op0=OP.max)
                    nc.vector.tensor_tensor(out=xm3, in0=xm3, in1=sc1bc, op=OP.mult)
                    nc.vector.tensor_tensor(out=xmega, in0=xmega, in1=A1m, op=OP.min)
                    for q2 in range(CHUNK // GRP):
                        ymega = psum.tile([128, GRP * 256], F32, tag="ymega",
                                          name=f"ym_{sgi}_{chi}_{q2}", bufs=2)
                        ymg = ymega.rearrange("p (g c) -> p g c", c=256)
                        for ii in range(GRP):
                            i = t0 + q2 * GRP + ii
                            k = q2 * GRP + ii
                            nc.tensor.matmul(xTm[:, ii * 128:(ii + 1) * 128],
                                             lhsT=xmega[:, k * 128:(k + 1) * 128],
                                             rhs=i128, start=True, stop=True)
                        xT_sb = grpw.tile([128, GRP * 128], BF16, tag="xT_sb",
                                          name=f"xTs_{sgi}_{chi}_{q2}", bufs=sgrp // GRP + 2)
                        nc.vector.tensor_copy(out=xT_sb, in_=xTm)
                        xT_sbs.append(xT_sb)
                        for ii in range(GRP):
                            i = t0 + q2 * GRP + ii
                            y_ap = ymega[:, ii * 256: ii * 256 + YW]
                            nc.tensor.matmul(y_ap, lhsT=xT_sb[:, ii * 128:(ii + 1) * 128],
                                             rhs=ywx, start=True, stop=False)
                            nc.tensor.matmul(y_ap, lhsT=jsl(i, 0), rhs=ywj[0:16, :],
                                             start=False, stop=True)
                        for ii in range(GRP):
                            i = t0 + q2 * GRP + ii
                            nc.vector.bn_stats(out=yst6[:, i, :], in_=ymg[:, ii, 0:DIN])
                        nc.vector.tensor_scalar(
                            out=mu2g[:, t0 + q2 * GRP:t0 + (q2 + 1) * GRP],
                            in0=ymg[:, :, DIN], scalar1=1.0 / HID, scalar2=None, op0=OP.mult)

                # ---- LN2 smalls (batched) ----
                def sumsq(st6, cnt, tag):
                    cv = sg.tile([128, sgrp], F32, tag=f"{tag}cv", name=f"{tag}cv_{sgi}")
                    nc.vector.tensor_tensor(out=cv, in0=st6[:, :, 2], in1=st6[:, :, 5], op=OP.add)
                    ms = sg.tile([128, sgrp], F32, tag=f"{tag}ms", name=f"{tag}ms_{sgi}")
                    nc.vector.tensor_tensor(out=ms, in0=st6[:, :, 1], in1=st6[:, :, 4], op=OP.add)
                    dd = sg.tile([128, sgrp], F32, tag=f"{tag}dd", name=f"{tag}dd_{sgi}")
                    nc.vector.tensor_tensor(out=dd, in0=st6[:, :, 1], in1=st6[:, :, 4], op=OP.subtract)
                    nc.vector.tensor_tensor(out=dd, in0=dd, in1=dd, op=OP.mult)
                    nc.vector.tensor_tensor(out=ms, in0=ms, in1=ms, op=OP.mult)
                    nc.vector.tensor_tensor(out=dd, in0=dd, in1=ms, op=OP.add)
                    nc.vector.scalar_tensor_tensor(out=cv, in0=dd, scalar=cnt / 4.0,
                                                   in1=cv, op0=OP.mult, op1=OP.add)
                    return cv
                sqA = sumsq(yst6, DIN, "sqA")
                m2sq = sg.tile([128, sgrp], F32, tag="m2sq", name=f"m2sq_{sgi}")
                nc.vector.tensor_tensor(out=m2sq, in0=mu2g, in1=mu2g, op=OP.mult)
                nc.vector.tensor_scalar(out=m2sq, in0=m2sq, scalar1=EPS, scalar2=None,
                                        op0=OP.subtract)
                nc.vector.scalar_tensor_tensor(out=wboth[:, sgrp:2 * sgrp], in0=sqA,
                                               scalar=1.0 / HID, in1=m2sq,
                                               op0=OP.mult, op1=OP.subtract)
                inv2g = _newton_rsqrt(nc, sg, wboth[:, sgrp:2 * sgrp], sgrp)
                q2 = sg.tile([128, sgrp], F32, tag="q2", name=f"q2_{sgi}")
                nc.vector.tensor_tensor(out=q2, in0=mu2g, in1=inv2g, op=OP.mult)
                biasA2g = sg.tile([128, sgrp], F32, tag="biasA2g", name=f"bA2_{sgi}")
                nc.vector.tensor_scalar(out=biasA2g, in0=q2, scalar1=-1.0, scalar2=LNLA,
                                        op0=OP.mult, op1=OP.add)
                biasB2g = sg.tile([128, sgrp], F32, tag="biasB2g", name=f"bB2_{sgi}")
                nc.vector.tensor_scalar(out=biasB2g, in0=q2, scalar1=-LAM, scalar2=None,
                                        op0=OP.mult)
                sc2g = sg.tile([128, sgrp], F32, tag="sc2g", name=f"sc2g_{sgi}")
                nc.vector.tensor_scalar(out=sc2g, in0=inv2g, scalar1=LAM, scalar2=None,
                                        op0=OP.mult)

                # ---------- phase Z: per 4-tile group ----------
                for q in range(sgrp // GRP):
                    xT_sb = xT_sbs[q]
                    zmega = psum.tile([128, GRP * HID], F32, tag="zmega",
                                      name=f"zm_{sgi}_{q}", bufs=2)
                    A2m = grpw.tile([128, GRP * HID], BF16, tag="A2m", name=f"A2m_{sgi}_{q}")
                    B2m = grpw.tile([128, GRP * HID], BF16, tag="B2m", name=f"B2m_{sgi}_{q}")
                    for ii in range(GRP):
                        i = q * GRP + ii
                        nc.tensor.matmul(zmega[:, ii * HID:(ii + 1) * HID],
                                         lhsT=xT_sb[:, ii * 128:(ii + 1) * 128],
                                         rhs=w1x, start=True, stop=False)
                        nc.tensor.matmul(zmega[:, ii * HID:(ii + 1) * HID],
                                         lhsT=jsl(i, 0), rhs=w1j[0:16, :],
                                         start=False, stop=True)
                    for ii in range(GRP):
                        i = q * GRP + ii
                        zsl = zmega[:, ii * HID:(ii + 1) * HID]
                        nc.scalar.activation(
                            out=A2m[:, ii * HID:(ii + 1) * HID], in_=zsl, func=AF.Exp,
                            bias=biasA2g[:, i:i + 1], scale=inv2g[:, i:i + 1])
                        nc.scalar.activation(
                            out=B2m[:, ii * HID:(ii + 1) * HID], in_=zsl, func=AF.Relu,
                            bias=biasB2g[:, i:i + 1], scale=sc2g[:, i:i + 1])
                    nc.vector.tensor_scalar(out=A2m, in0=A2m, scalar1=LA, scalar2=None,
                                            op0=OP.subtract)
                    x2m = grpw.tile([128, GRP * HID], BF16, tag="x2m", name=f"x2m_{sgi}_{q}")
                    nc.vector.tensor_tensor(out=x2m, in0=B2m, in1=A2m, op=OP.min)
                    for ii in range(GRP):
                        for cc in range(4):
                            nc.tensor.matmul(
                                zmega[:, ii * HID + cc * 128: ii * HID + (cc + 1) * 128],
                                lhsT=x2m[:, ii * HID + cc * 128: ii * HID + (cc + 1) * 128],
                                rhs=i128, start=True, stop=True)
                    x2T_sb = grpw.tile([128, GRP * HID], BF16, tag="x2T_sb",
                                       name=f"x2T_{sgi}_{q}")
                    nc.vector.tensor_copy(out=x2T_sb, in_=zmega)
                    m2out = psum.tile([128, GRP * JNT], F32, tag="m2out",
                                      name=f"m2o_{sgi}_{q}", bufs=1)
                    for ii in range(GRP):
                        for cc in range(4):
                            nc.tensor.matmul(
                                m2out[:, ii * JNT:(ii + 1) * JNT],
                                lhsT=x2T_sb[:, ii * HID + cc * 128: ii * HID + (cc + 1) * 128],
                                rhs=w2c[:, cc * JNT:(cc + 1) * JNT],
                                start=(cc == 0), stop=(cc == 3))
                    if with_b2:
                        nc.vector.tensor_tensor(out=m2out, in0=m2out, in1=b2c, op=OP.add)
                    tanhg = grpw.tile([128, GRP * JNT], F32, tag="tanhg",
                                      name=f"th_{sgi}_{q}")
                    nc.scalar.activation(out=tanhg, in_=m2out, func=AF.Tanh)
                    gt0 = st0 + q * GRP
                    nc.vector.tensor_scalar(
                        out=outres[:, gt0 * JNT:(gt0 + GRP) * JNT], in0=tanhg,
                        scalar1=TAU, scalar2=None, op0=OP.mult)

            nc.sync.dma_start(out=out_e[:, :], in_=outres)

    nc.compile()
    return nc


_CACHE = {}


def _get_nc(rpc, idxn, with_b2):
    key = (rpc, idxn, with_b2)
    if key not in _CACHE:
        _CACHE[key] = build(rpc, idxn, with_b2)
    return _CACHE[key]


def _prep_core(feats, jcat_T, core, rpc):
    tiles = rpc // 128
    f = feats[core * rpc:(core + 1) * rpc]
    f_t = np.ascontiguousarray(
        f.reshape(tiles, 128, C).transpose(1, 0, 2).reshape(128, tiles * C)
    ).astype(NP_BF16)
    j_t = np.ascontiguousarray(jcat_T[:, core * rpc:(core + 1) * rpc]).astype(NP_BF16)
    return f_t, j_t


def _mask_indices(coords, rpc, idxn):
    """Per-core ([128, idxn] int16 indices, [rpc] 0/1 mask)."""
    u = np.unique(coords)
    idxs, masks = [], []
    for core in range(N_CORES):
        lo, hi = core * rpc, (core + 1) * rpc
        lu = u[(u >= lo) & (u < hi)] - lo
        m = np.zeros(rpc, np.float32)
        m[lu] = 1.0
        p = lu & 127
        f = lu >> 7
        idx = np.full((128, idxn), -1, dtype=np.int16)
        order = np.argsort(p, kind="stable")
        ps, fs = p[order], f[order]
        counts = np.bincount(ps, minlength=128)
        assert counts.max(initial=0) <= idxn, f"bucket overflow {counts.max()}"
        start = 0
        for part in range(128):
            cnt = counts[part]
            idx[part, :cnt] = fs[start:start + cnt]
            start += cnt
        idxs.append(idx)
        masks.append(m)
    return idxs, masks


def kernel(feats, coords, jnt_pos, jnt_goal, weights,
           ln1_g, ln1_b, W1, b1, ln2_g, ln2_b, W2, b2):
    return _run(feats, coords, jnt_pos, jnt_goal, weights,
                ln1_g, ln1_b, W1, b1, ln2_g, ln2_b, W2, b2, rpc=RPC)


def _run(feats, coords, jnt_pos, jnt_goal, weights,
         ln1_g, ln1_b, W1, b1, ln2_g, ln2_b, W2, b2, rpc):
    n_all = rpc * N_CORES
    feats = np.asarray(feats, dtype=np.float32)
    coords = np.asarray(coords, dtype=np.int32)
    jnt_pos = np.asarray(jnt_pos, dtype=np.float32)
    jnt_goal = np.asarray(jnt_goal, dtype=np.float32)
    weights = np.asarray(weights, dtype=np.float32)
    ln1_g = np.asarray(ln1_g, dtype=np.float32)
    ln1_b = np.asarray(ln1_b, dtype=np.float32)
    W1 = np.asarray(W1, dtype=np.float32)
    b1 = np.asarray(b1, dtype=np.float32)
    ln2_g = np.asarray(ln2_g, dtype=np.float32)
    ln2_b = np.asarray(ln2_b, dtype=np.float32)
    W2 = np.asarray(W2, dtype=np.float32)
    b2 = np.asarray(b2, dtype=np.float32)

    assert feats.shape == (n_all, C) and coords.shape == (n_all,)
    assert np.allclose(ln1_b, 0.0), "ln1_b != 0 unsupported"
    assert np.allclose(ln1_g, 1.0), "ln1_g != 1 unsupported"
    assert np.allclose(ln2_g, 1.0) and np.allclose(ln2_b, 0.0), "ln2 affine unsupported"

    with_b2 = not np.allclose(b2, 0.0)
    nc = _get_nc(rpc, IDXN, with_b2)

    midxs, masks = _mask_indices(coords, rpc, IDXN)

    W1x = W1[13:141] * ln1_g[:, None]          # [128, 512]
    w2c = np.ascontiguousarray(
        W2.reshape(4, 128, JNT).transpose(1, 0, 2).reshape(128, 4 * JNT))

    const_map = {
        "ident": np.eye(128, dtype=np.float32).astype(NP_BF16),
        "w1x": W1x.astype(NP_BF16),
        "w2c": w2c.astype(NP_BF16),
    }
    if with_b2:
        const_map["b2c"] = np.tile(b2, (128, GRP)).astype(np.float32)

    # bf16-exact mask correction: the device writes x = bf16(-LA) for masked
    # rows; rows 14/15 carry colsum(W1x_bf16) split into bf16 coarse+residual.
    la_dev = float(np.float32(LA).astype(NP_BF16))
    W1x_bf = W1x.astype(NP_BF16).astype(np.float64)
    S = W1x_bf.sum(axis=0)
    S_hi = S.astype(np.float32).astype(NP_BF16).astype(np.float64)
    S_lo = (S - S_hi).astype(np.float32)
    W1j = np.zeros((DJ, HID), np.float32)
    W1j[:13] = W1[:13]
    W1j[13] = b1
    W1j[14] = S_hi
    W1j[15] = S_lo
    W1jq = W1j.astype(NP_BF16).astype(np.float64)
    W1aug = np.vstack([W1jq, W1x_bf])   # [144, 512] bf16-consistent
    G = W1aug @ W1aug.T
    evals, evecs = np.linalg.eigh(G)
    Weig = evecs * np.sqrt(np.maximum(evals, 0.0))[None, :]   # [144, 144]
    w1s = W1aug.sum(axis=1)
    yWa = np.concatenate([Weig, w1s[:, None]], axis=1).astype(np.float32)  # [144,145]
    w1j2 = np.zeros((64, HID), np.float32)
    w1j2[0:DJ] = W1j
    w1j2[32:32 + DJ] = W1j
    ywj2 = np.zeros((64, YW), np.float32)
    ywj2[0:DJ] = yWa[0:DJ]
    ywj2[32:32 + DJ] = yWa[0:DJ]
    const_map["w1j"] = w1j2.astype(NP_BF16)
    const_map["ywj"] = ywj2.astype(NP_BF16)
    const_map["ywx"] = yWa[DJ:DIN].astype(NP_BF16)

    in_maps = []
    for core in range(N_CORES):
        m = dict(const_map)

        jcat_T = np.zeros((64, rpc), np.float32)
        r0 = core * rpc
        for s0 in (0, 32):
            jcat_T[s0 + 0:s0 + JNT] = jnt_pos[r0:r0 + rpc].T
            jcat_T[s0 + JNT:s0 + 2 * JNT] = jnt_goal[r0:r0 + rpc].T
            jcat_T[s0 + 2 * JNT:s0 + 13] = weights[r0:r0 + rpc].T
            jcat_T[s0 + 13] = 1.0
            jcat_T[s0 + 14] = la_dev * (1.0 - masks[core])
            jcat_T[s0 + 15] = la_dev * (1.0 - masks[core])

        tiles = rpc // 128
        f = feats[r0:r0 + rpc]
        m["feats_t"] = np.ascontiguousarray(
            f.reshape(tiles, 128, C).transpose(1, 0, 2).reshape(128, tiles * C)
        ).astype(NP_BF16)
        m["jcat_t"] = np.ascontiguousarray(jcat_T).astype(NP_BF16)
        m["midx"] = midxs[core]
        in_maps.append(m)

    global LAST_EXEC_NS, LAST_TRACE_DIR
    import tempfile
    kw = {}
    if TRACE:
        kw = dict(trace=True, tmpdir=tempfile.mkdtemp(prefix="actor_trace_"))
    res = run_bass_kernel_spmd(nc, in_maps, core_ids=list(range(N_CORES)), **kw)
    LAST_EXEC_NS = res.exec_time_ns
    LAST_TRACE_DIR = kw.get("tmpdir")

    tiles = rpc // 128
    out = np.empty((n_all, JNT), np.float32)
    for core in range(N_CORES):
        o = res.results[core]["out_t"]
        o = o.reshape(128, tiles, JNT).transpose(1, 0, 2).reshape(rpc, JNT)
        out[core * rpc:(core + 1) * rpc] = o
    return out
